# revision 6
# baseline (speedup 1.0000x reference)
"""BitLinear TRN2 kernel: out = layernorm(x) @ sign(w).T + bias.

Tensor-parallel over out_features, 8 cores. Transfer-lean contract:
each core ships only its 1/8 token shard of x^T (bf16) plus its
[4096, 2048] transposed sign-weight shard (fp8); the full x^T is
assembled ON DEVICE by chunked AllGathers over NeuronLink (4 chunks,
pipelined so matmuls start after the first). Output returns as bf16
and the host concats/casts. Per-call axon traffic drops from ~2.2GB
(baseline) to ~0.64GB.

All layout/elementwise prep rides the host (sign, transposes, bf16/fp8
casts, LN stats); the device does the O(T*D*O) einsum at the PE bf16
roofline plus a 2-op DVE evict. LN is folded around the matmul on raw
x:  out[t,o] = (x@bw.T)[t,o] * inv_t + a_t * S[o] + bias[o]
with S[o] = sum_d bw[o,d], inv_t = 1/(std_t+eps), a_t = -mu_t*inv_t.
inv/a arrive per-token from the host arranged [128, 64] (partition =
token%128, col = token chunk); a_t*S[o]+bias[o] is built once per token
chunk (cb), and each PSUM eviction is psum*inv (+cb) straight to bf16.

Device layout: transposed sign weights live resident in SBUF as bf16
[128, 32, 2048] (128KB/partition), cast once from the fp8 shipment.
Gathered x^T streams in 256-token superblocks [128, 32, 256] bf16 (one
3D-AP DMA each from the gather-chunk DRAM tensor). Matmuls: stationary
= x^T k-tile [128, 128], moving = weight slice [128, 512] (one fp32
PSUM bank), 32-deep k accumulation; 8192 MMs/core ~= the 78.6 TF/s
bf16 roofline. No PE transposes, no on-device stats, everything
PE-side is bf16.
"""

import os

import numpy as np
import ml_dtypes

import concourse.bass as bass
import concourse.tile as tile
from concourse import bacc
from concourse import mybir
from concourse.bass_utils import run_bass_kernel_spmd

F32 = mybir.dt.float32
BF16 = mybir.dt.bfloat16
FP8 = mybir.dt.float8e4

NP_BF16 = ml_dtypes.bfloat16
NP_FP8 = ml_dtypes.float8_e4m3

T, D, O_FULL, NCORES = 8192, 4096, 16384, 8
O = O_FULL // NCORES  # 2048 out-features per core
TS = T // NCORES  # 1024 tokens shipped per core
EPS = 1e-5

KT = D // 128  # 32 k-tiles
MC = T // 128  # 64 token chunks
NG = 4  # x^T AllGather chunks (pipelined with compute)
GT = TS // NG  # 256 local tokens per gather chunk
SUP = 256  # tokens per superblock == GT
CPS = SUP // 128  # 2 chunks per superblock
WN = 512  # moving free width (one PSUM bank at fp32)
NWCH = O // WN  # 4 out chunks

MODE = os.environ.get("BITLIN_MODE", "bf16_ag")


def _build():
    nc = bacc.Bacc("TRN2", target_bir_lowering=False, debug=False)
    xt_d = nc.declare_dram_parameter("xt", [D, TS], BF16, isOutput=False)
    w8_d = nc.declare_dram_parameter("w8", [D, O], FP8, isOutput=False)
    s_d = nc.declare_dram_parameter("srow", [O], BF16, isOutput=False)
    b_d = nc.declare_dram_parameter("brow", [O], BF16, isOutput=False)
    iv_d = nc.declare_dram_parameter("iv", [128, MC], F32, isOutput=False)
    aa_d = nc.declare_dram_parameter("aa", [128, MC], F32, isOutput=False)
    out_d = nc.declare_dram_parameter("out", [T, O], BF16, isOutput=True)

    with tile.TileContext(nc) as tc:
        with (
            tc.tile_pool(name="dram", bufs=1, space="DRAM") as dram,
            tc.tile_pool(name="singles", bufs=1) as singles,
            tc.tile_pool(name="w8p", bufs=2) as w8p,
            tc.tile_pool(name="xtp", bufs=2) as xtp,
            tc.tile_pool(name="cbp", bufs=2) as cbp,
            tc.tile_pool(name="evp", bufs=4) as evp,
            tc.tile_pool(name="mmp", bufs=3, space="PSUM") as mmp,
        ):
            wt = singles.tile([128, KT, O], BF16)  # resident sign(w)^T
            s_bc = singles.tile([128, O], BF16)  # S broadcast across partitions
            b_bc = singles.tile([128, O], BF16)  # bias broadcast
            iv_sb = singles.tile([128, MC], F32)
            aa_sb = singles.tile([128, MC], F32)

            # ---- x^T shard -> bounce -> chunked AllGather over the 8 cores
            gxs = []
            for j in range(NG):
                ib = dram.tile([D, GT], BF16, tag=f"ib{j}", name=f"ib{j}")
                gx = dram.tile(
                    [NCORES * D, GT],
                    BF16,
                    tag=f"gx{j}",
                    name=f"gx{j}",
                    addr_space="Shared",
                )
                nc.gpsimd.dma_start(out=ib[:], in_=xt_d[:, j * GT : (j + 1) * GT])
                nc.gpsimd.collective_compute(
                    "AllGather",
                    mybir.AluOpType.bypass,
                    replica_groups=[list(range(NCORES))],
                    ins=[ib.opt()],
                    outs=[gx.opt()],
                )
                gxs.append(gx)

            # ---- prep: tiny stat/bias loads + partition-broadcast rows ----
            nc.gpsimd.dma_start(out=iv_sb[:], in_=iv_d[:, :])
            nc.gpsimd.dma_start(out=aa_sb[:], in_=aa_d[:, :])
            sap = s_d[:]
            nc.gpsimd.dma_start(
                out=s_bc[:],
                in_=bass.AP(tensor=sap.tensor, offset=sap.offset, ap=[[0, 128]] + sap.ap),
            )
            bap = b_d[:]
            nc.gpsimd.dma_start(
                out=b_bc[:],
                in_=bass.AP(tensor=bap.tensor, offset=bap.offset, ap=[[0, 128]] + bap.ap),
            )

            # ---- prep: land fp8 sign-weight stripes, upcast into resident wt
            for k in range(KT):
                w8 = w8p.tile([128, O], FP8, tag="w8")
                nc.gpsimd.dma_start(out=w8[:], in_=w8_d[k * 128 : (k + 1) * 128, :])
                nc.vector.tensor_copy(out=wt[:, k, :], in_=w8[:])

            # ---- main: stream gathered x^T superblocks, matmul, fused evict
            for j in range(NG):
                gx = gxs[j]
                for cg in range(NCORES):
                    # global tokens [cg*TS + j*GT, +SUP) live in gx rows cg*D..
                    xtb = xtp.tile([128, KT, SUP], BF16, tag="xtb")
                    gap = gx[:]
                    nc.gpsimd.dma_start(
                        out=xtb[:],
                        in_=bass.AP(
                            tensor=gap.tensor,
                            offset=gap.offset + cg * D * GT,
                            ap=[[GT, 128], [128 * GT, KT], [1, SUP]],
                        ),
                    )
                    for c in range(CPS):
                        m = (cg * TS + j * GT) // 128 + c
                        tsl = slice(c * 128, (c + 1) * 128)
                        # cb[p, o] = a_t * S[o] + bias[o] for this token chunk
                        cb = cbp.tile([128, O], BF16, tag="cb")
                        nc.vector.tensor_scalar_mul(
                            cb[:], s_bc[:], aa_sb[:, m : m + 1]
                        )
                        nc.vector.tensor_add(cb[:], cb[:], b_bc[:])
                        for oc in range(NWCH):
                            osl = slice(oc * WN, (oc + 1) * WN)
                            pm = mmp.tile([128, WN], F32, tag="mm")
                            for k in range(KT):
                                nc.tensor.matmul(
                                    pm[:],
                                    xtb[:, k, tsl],
                                    wt[:, k, osl],
                                    start=(k == 0),
                                    stop=(k == KT - 1),
                                )
                            ev = evp.tile([128, WN], BF16, tag="ev")
                            nc.vector.tensor_scalar_mul(
                                ev[:], pm[:], iv_sb[:, m : m + 1]
                            )
                            nc.vector.tensor_add(ev[:], ev[:], cb[:, osl])
                            nc.gpsimd.dma_start(
                                out=out_d[m * 128 : (m + 1) * 128, osl], in_=ev[:]
                            )
    nc.compile()
    return nc


_NC_CACHE = {}
LAST_RESULTS = None


def kernel(x, weight, bias):
    global LAST_RESULTS
    x = np.asarray(x, dtype=np.float32)
    weight = np.asarray(weight, dtype=np.float32)
    bias = np.asarray(bias, dtype=np.float32)

    # LN stats folded to a per-token affine: out = (x@bw.T)*inv + a*S + b
    mu = x.mean(axis=1, dtype=np.float64)
    sd = np.sqrt(x.var(axis=1, ddof=1, dtype=np.float64))
    inv = (1.0 / (sd + EPS)).astype(np.float32)
    aa = (-mu * inv).astype(np.float32)
    iv_t = np.ascontiguousarray(inv.reshape(MC, 128).T)  # [128, chunk]
    aa_t = np.ascontiguousarray(aa.reshape(MC, 128).T)

    xT = x.T.astype(NP_BF16)  # [D, T] bf16, C-contiguous
    ws = np.sign(weight)  # [O_FULL, D] f32 in {-1, 0, +1}
    S = ws.sum(axis=1).astype(NP_BF16)  # [O_FULL]
    b16 = bias.astype(NP_BF16)
    wsT8 = ws.T.astype(NP_FP8)  # [D, O_FULL] fp8 (+-1 exact)

    if "nc" not in _NC_CACHE:
        _NC_CACHE["nc"] = _build()
    nc = _NC_CACHE["nc"]

    in_maps = []
    for i in range(NCORES):
        in_maps.append(
            {
                "xt": xT[:, i * TS : (i + 1) * TS],
                "w8": wsT8[:, i * O : (i + 1) * O],
                "srow": S[i * O : (i + 1) * O],
                "brow": b16[i * O : (i + 1) * O],
                "iv": iv_t,
                "aa": aa_t,
            }
        )
    res = run_bass_kernel_spmd(nc, in_maps, list(range(NCORES)))
    LAST_RESULTS = res
    out = np.empty((T, O_FULL), np.float32)
    for i in range(NCORES):
        out[:, i * O : (i + 1) * O] = res.results[i]["out"]
    return out


# revision 9
# speedup vs baseline: 1.5898x; 1.5898x over previous
"""BitLinear TRN2 kernel: out = layernorm(x) @ sign(w).T + bias.

Tensor-parallel over out_features, 8 cores. Transfer-lean contract:
each core ships only its 1/8 token shard of x^T (bf16) plus its
[4096, 2048] transposed sign-weight shard (fp8); the full x^T is
assembled ON DEVICE by chunked AllGathers over NeuronLink (4 chunks,
pipelined so matmuls start after the first). Output returns as bf16
and the host concats/casts. Per-call axon traffic drops from ~2.2GB
(baseline) to ~0.64GB.

All layout/elementwise prep rides the host (sign, transposes, bf16/fp8
casts, LN stats); the device does the O(T*D*O) einsum at the PE bf16
roofline plus a 2-op DVE evict. LN is folded around the matmul on raw
x:  out[t,o] = (x@bw.T)[t,o] * inv_t + a_t * S[o] + bias[o]
with S[o] = sum_d bw[o,d], inv_t = 1/(std_t+eps), a_t = -mu_t*inv_t.
inv/a arrive per-token from the host arranged [128, 64] (partition =
token%128, col = token chunk); a_t*S[o]+bias[o] is built once per token
chunk (cb), and each PSUM eviction is psum*inv (+cb) straight to bf16.

Device layout: transposed sign weights live resident in SBUF as bf16
[128, 32, 2048] (128KB/partition), cast once from the fp8 shipment.
Gathered x^T streams in 256-token superblocks [128, 32, 256] bf16 (one
3D-AP DMA each from the gather-chunk DRAM tensor). Matmuls: stationary
= x^T k-tile [128, 128], moving = weight slice [128, 512] (one fp32
PSUM bank), 32-deep k accumulation; 8192 MMs/core ~= the 78.6 TF/s
bf16 roofline. No PE transposes, no on-device stats, everything
PE-side is bf16.
"""

import os

import numpy as np
import ml_dtypes

import concourse.bass as bass
import concourse.tile as tile
from concourse import bacc
from concourse import mybir
from concourse.bass_utils import run_bass_kernel_spmd

F32 = mybir.dt.float32
BF16 = mybir.dt.bfloat16
FP8 = mybir.dt.float8e4

NP_BF16 = ml_dtypes.bfloat16
NP_FP8 = ml_dtypes.float8_e4m3

T, D, O_FULL, NCORES = 8192, 4096, 16384, 8
O = O_FULL // NCORES  # 2048 out-features per core
TS = T // NCORES  # 1024 tokens shipped per core
EPS = 1e-5

KT = D // 128  # 32 k-tiles
MC = T // 128  # 64 token chunks
NG = 4  # x^T AllGather chunks (pipelined with compute)
GT = TS // NG  # 256 local tokens per gather chunk
SUP = 256  # tokens per superblock == GT
CPS = SUP // 128  # 2 chunks per superblock
WN = 512  # moving free width (one PSUM bank at fp32)
NWCH = O // WN  # 4 out chunks

MODE = os.environ.get("BITLIN_MODE", "bf16_ag")


def _build():
    nc = bacc.Bacc("TRN2", target_bir_lowering=False, debug=False)
    xt_d = nc.declare_dram_parameter("xt", [D, TS], BF16, isOutput=False)
    w8_d = nc.declare_dram_parameter("w8", [D, O], FP8, isOutput=False)
    s_d = nc.declare_dram_parameter("srow", [O], BF16, isOutput=False)
    b_d = nc.declare_dram_parameter("brow", [O], BF16, isOutput=False)
    iv_d = nc.declare_dram_parameter("iv", [128, MC], F32, isOutput=False)
    aa_d = nc.declare_dram_parameter("aa", [128, MC], F32, isOutput=False)
    out_d = nc.declare_dram_parameter("out", [T, O], BF16, isOutput=True)

    with tile.TileContext(nc) as tc:
        with (
            tc.tile_pool(name="dram", bufs=1, space="DRAM") as dram,
            tc.tile_pool(name="singles", bufs=1) as singles,
            tc.tile_pool(name="w8p", bufs=2) as w8p,
            tc.tile_pool(name="xtp", bufs=2) as xtp,
            tc.tile_pool(name="cbp", bufs=2) as cbp,
            tc.tile_pool(name="evp", bufs=4) as evp,
            tc.tile_pool(name="mmp", bufs=3, space="PSUM") as mmp,
        ):
            wt = singles.tile([128, KT, O], BF16)  # resident sign(w)^T
            s_bc = singles.tile([128, O], BF16)  # S broadcast across partitions
            b_bc = singles.tile([128, O], BF16)  # bias broadcast
            iv_sb = singles.tile([128, MC], F32)
            aa_sb = singles.tile([128, MC], F32)

            # ---- x^T shard -> bounce -> chunked AllGather over the 8 cores
            gxs = []
            for j in range(NG):
                ib = dram.tile([D, GT], BF16, tag=f"ib{j}", name=f"ib{j}")
                gx = dram.tile(
                    [NCORES * D, GT],
                    BF16,
                    tag=f"gx{j}",
                    name=f"gx{j}",
                    addr_space="Shared",
                )
                nc.gpsimd.dma_start(out=ib[:], in_=xt_d[:, j * GT : (j + 1) * GT])
                nc.gpsimd.collective_compute(
                    "AllGather",
                    mybir.AluOpType.bypass,
                    replica_groups=[list(range(NCORES))],
                    ins=[ib.opt()],
                    outs=[gx.opt()],
                )
                gxs.append(gx)

            # ---- prep: tiny stat/bias loads + partition-broadcast rows ----
            nc.gpsimd.dma_start(out=iv_sb[:], in_=iv_d[:, :])
            nc.gpsimd.dma_start(out=aa_sb[:], in_=aa_d[:, :])
            sap = s_d[:]
            nc.gpsimd.dma_start(
                out=s_bc[:],
                in_=bass.AP(tensor=sap.tensor, offset=sap.offset, ap=[[0, 128]] + sap.ap),
            )
            bap = b_d[:]
            nc.gpsimd.dma_start(
                out=b_bc[:],
                in_=bass.AP(tensor=bap.tensor, offset=bap.offset, ap=[[0, 128]] + bap.ap),
            )

            # ---- prep: land fp8 sign-weight stripes, upcast into resident wt
            for k in range(KT):
                w8 = w8p.tile([128, O], FP8, tag="w8")
                nc.gpsimd.dma_start(out=w8[:], in_=w8_d[k * 128 : (k + 1) * 128, :])
                nc.vector.tensor_copy(out=wt[:, k, :], in_=w8[:])

            # ---- main: stream gathered x^T superblocks, matmul, fused evict
            for j in range(NG):
                gx = gxs[j]
                for cg in range(NCORES):
                    # global tokens [cg*TS + j*GT, +SUP) live in gx rows cg*D..
                    xtb = xtp.tile([128, KT, SUP], BF16, tag="xtb")
                    gap = gx[:]
                    nc.gpsimd.dma_start(
                        out=xtb[:],
                        in_=bass.AP(
                            tensor=gap.tensor,
                            offset=gap.offset + cg * D * GT,
                            ap=[[GT, 128], [128 * GT, KT], [1, SUP]],
                        ),
                    )
                    for c in range(CPS):
                        m = (cg * TS + j * GT) // 128 + c
                        tsl = slice(c * 128, (c + 1) * 128)
                        # cb[p, o] = a_t * S[o] + bias[o] for this token chunk
                        cb = cbp.tile([128, O], BF16, tag="cb")
                        nc.vector.tensor_scalar_mul(
                            cb[:], s_bc[:], aa_sb[:, m : m + 1]
                        )
                        nc.vector.tensor_add(cb[:], cb[:], b_bc[:])
                        for oc in range(NWCH):
                            osl = slice(oc * WN, (oc + 1) * WN)
                            pm = mmp.tile([128, WN], F32, tag="mm")
                            for k in range(KT):
                                nc.tensor.matmul(
                                    pm[:],
                                    xtb[:, k, tsl],
                                    wt[:, k, osl],
                                    start=(k == 0),
                                    stop=(k == KT - 1),
                                )
                            ev = evp.tile([128, WN], BF16, tag="ev")
                            nc.vector.tensor_scalar_mul(
                                ev[:], pm[:], iv_sb[:, m : m + 1]
                            )
                            nc.vector.tensor_add(ev[:], ev[:], cb[:, osl])
                            nc.gpsimd.dma_start(
                                out=out_d[m * 128 : (m + 1) * 128, osl], in_=ev[:]
                            )
    nc.compile()
    return nc


_NC_CACHE = {}
LAST_RESULTS = None


def kernel(x, weight, bias):
    global LAST_RESULTS
    import time as _time

    _tv = os.environ.get("BITLIN_TIME", "0") == "1"
    _t0 = _time.time()
    x = np.asarray(x, dtype=np.float32)
    weight = np.asarray(weight, dtype=np.float32)
    bias = np.asarray(bias, dtype=np.float32)

    # LN stats folded to a per-token affine: out = (x@bw.T)*inv + a*S + b
    mu = x.mean(axis=1, dtype=np.float64)
    sd = np.sqrt(x.var(axis=1, ddof=1, dtype=np.float64))
    inv = (1.0 / (sd + EPS)).astype(np.float32)
    aa = (-mu * inv).astype(np.float32)
    iv_t = np.ascontiguousarray(inv.reshape(MC, 128).T)  # [128, chunk]
    aa_t = np.ascontiguousarray(aa.reshape(MC, 128).T)

    xT = x.T.astype(NP_BF16)  # [D, T] bf16, C-contiguous
    ws = np.sign(weight)  # [O_FULL, D] f32 in {-1, 0, +1}
    S = ws.sum(axis=1).astype(NP_BF16)  # [O_FULL]
    b16 = bias.astype(NP_BF16)
    wsT8 = ws.T.astype(NP_FP8)  # [D, O_FULL] fp8 (+-1 exact)

    if _tv:
        print(f"[bitlin] host prep: {_time.time() - _t0:.2f}s", flush=True)
    if "nc" not in _NC_CACHE:
        _NC_CACHE["nc"] = _build()
    nc = _NC_CACHE["nc"]
    _t1 = _time.time()

    in_maps = []
    for i in range(NCORES):
        in_maps.append(
            {
                "xt": xT[:, i * TS : (i + 1) * TS],
                "w8": wsT8[:, i * O : (i + 1) * O],
                "srow": S[i * O : (i + 1) * O],
                "brow": b16[i * O : (i + 1) * O],
                "iv": iv_t,
                "aa": aa_t,
            }
        )
    res = run_bass_kernel_spmd(nc, in_maps, list(range(NCORES)))
    LAST_RESULTS = res
    if _tv:
        print(f"[bitlin] run_bass: {_time.time() - _t1:.2f}s", flush=True)
    out = np.empty((T, O_FULL), np.float32)
    for i in range(NCORES):
        out[:, i * O : (i + 1) * O] = res.results[i]["out"]
    if _tv:
        print(f"[bitlin] out assemble: {_time.time() - _t1:.2f}s cum", flush=True)
    return out
